# revision 1
# baseline (speedup 1.0000x reference)
"""MultiHeadGAT layer on 8 Trainium2 NeuronCores.

Strategy (graph/data parallel, dst-sharded):
  - Nodes are partitioned into 8 contiguous ranges (6250 per core); each
    core owns the output rows for its range.
  - Edges are routed (host-side, index bookkeeping only) to the core that
    owns their destination node, sorted by destination tile, and padded to
    a uniform per-tile chunk count M (chunks of 128 edges).
  - Each core computes (replicated) a node table
        tbl[n] = [ xl(n) (128) | s_src(n) (4) | s_dst(n) (4) ]
    where xl = x @ W_lin.T, s_src = xl . att_dst (per head),
    s_dst = xl . att_src (per head)  (the reference applies att_src to
    x_i=xl[dst] and att_dst to x_j=xl[src]).
  - Edge phase, per destination tile (128 nodes) and per 128-edge chunk:
    gather tbl rows by src (132 floats) and s_dst by dst (4 floats) via
    indirect DMA, alpha = lrelu(s_src + s_dst + edge_attr @ C), e = exp
    (no segment-max shift: alphas are bounded, softmax is shift-invariant),
    msg = e * xl_src, one-hot(dst) matmul accumulates [numerator | denom]
    into PSUM over the tile's chunks.
  - Epilogue per tile: divide, +bias, +residual, LayerNorm, ELU, store.

No collectives: cores are fully independent (params + x replicated).
"""

import math

import numpy as np

import concourse.bass as bass
import concourse.bacc as bacc
import concourse.mybir as mybir
from concourse.tile import TileContext
from concourse.masks import make_identity
from concourse.bass_utils import run_bass_kernel_spmd

F32 = mybir.dt.float32
I32 = mybir.dt.int32
AF = mybir.ActivationFunctionType
OP = mybir.AluOpType
AX = mybir.AxisListType

H, C = 4, 32
HC = H * C          # 128
IN_CH = 128
ED = 16             # edge_attr dim
NEG_SLOPE = 0.2
LN_EPS = 1e-5
P = 128             # partitions / tile rows / chunk size
KB = 8              # chunks per gather batch

FULL_CFG = dict(
    n_nodes=50000,
    n_cores=8,
    n_edges=1600000,
)


def derive_cfg(cfg):
    n, cores = cfg["n_nodes"], cfg["n_cores"]
    npc = n // cores                    # nodes per core (n divisible by cores)
    assert npc * cores == n
    tiles = math.ceil(npc / P)          # dst tiles per core
    npad = tiles * P
    nt_tbl = math.ceil(n / P)           # table tiles (covers all nodes)
    n_tbl = nt_tbl * P
    return dict(cfg, npc=npc, tiles=tiles, npad=npad, nt_tbl=nt_tbl, n_tbl=n_tbl)


# --------------------------------------------------------------------------
# host-side edge routing
# --------------------------------------------------------------------------

def host_prep(x, edge_index, edge_attr, W_lin, W_edge, att_src, att_dst,
              att_edge, bias, ln_gamma, ln_beta, cfg):
    cfg = derive_cfg(cfg)
    cores, npc, tiles = cfg["n_cores"], cfg["npc"], cfg["tiles"]
    n, n_tbl = cfg["n_nodes"], cfg["n_tbl"]

    src = np.asarray(edge_index[0], np.int64)
    dst = np.asarray(edge_index[1], np.int64)
    ea = np.asarray(edge_attr, np.float32)
    E = src.shape[0]

    core_of = dst // npc
    local = dst - core_of * npc
    tile_of = local // P
    key = core_of * tiles + tile_of
    order = np.argsort(key, kind="stable")
    key_s = key[order]
    counts = np.bincount(key_s, minlength=cores * tiles)
    M = max(1, int(math.ceil(counts.max() / P)))
    cap = M * P

    # slot within (core,tile) group for each sorted edge
    group_start = np.zeros(cores * tiles, np.int64)
    np.cumsum(counts[:-1], out=group_start[1:])
    slot = np.arange(E, dtype=np.int64) - group_start[key_s]
    flat = key_s * cap + slot           # destination slot in padded layout

    tot = cores * tiles * cap
    src_f = np.zeros(tot, np.int32)
    rel_f = np.full(tot, -1.0, np.float32)
    ea_f = np.zeros((tot, ED), np.float32)

    src_f[flat] = src[order].astype(np.int32)
    rel_f[flat] = (local[order] - tile_of[order] * P).astype(np.float32)
    ea_f[flat] = ea[order]

    # [cores, tiles, M, P(,ED)] -> [cores, tiles, P, M(,ED)]
    src_sw = src_f.reshape(cores, tiles, M, P).transpose(0, 1, 3, 2).copy()
    rel_sw = rel_f.reshape(cores, tiles, M, P).transpose(0, 1, 3, 2).copy()
    relT_sw = rel_f.reshape(cores, tiles, M, P).copy()
    ea_sw = (ea_f.reshape(cores, tiles, M, P, ED)
             .transpose(0, 1, 3, 2, 4).copy())

    x = np.asarray(x, np.float32)
    x_pad = np.zeros((n_tbl, IN_CH), np.float32)
    x_pad[:n] = x

    npad = cfg["npad"]
    xres = np.zeros((cores, npad, IN_CH), np.float32)
    for c in range(cores):
        xres[c, :npc] = x[c * npc:(c + 1) * npc]

    att_flat = dict(
        a_src=np.asarray(att_src, np.float32).reshape(HC, 1),
        a_dst=np.asarray(att_dst, np.float32).reshape(HC, 1),
        a_edge=np.asarray(att_edge, np.float32).reshape(HC, 1),
    )
    iota = np.arange(P, dtype=np.float32).reshape(1, P)
    iota_c = np.arange(P, dtype=np.float32).reshape(P, 1)

    in_maps = []
    for c in range(cores):
        in_maps.append(dict(
            x=x_pad,
            xres=np.ascontiguousarray(xres[c]),
            src_sw=np.ascontiguousarray(src_sw[c]),
            rel_sw=np.ascontiguousarray(rel_sw[c]),
            relT_sw=np.ascontiguousarray(relT_sw[c]),
            ea_sw=np.ascontiguousarray(ea_sw[c]),
            W_lin=np.asarray(W_lin, np.float32),
            W_edge=np.asarray(W_edge, np.float32),
            bias=np.asarray(bias, np.float32).reshape(1, HC),
            ln_gamma=np.asarray(ln_gamma, np.float32).reshape(1, HC),
            ln_beta=np.asarray(ln_beta, np.float32).reshape(1, HC),
            iota=iota,
            iota_c=iota_c,
            **att_flat,
        ))
    return in_maps, M, cfg


# --------------------------------------------------------------------------
# device program
# --------------------------------------------------------------------------

def build_program(M, cfg, num_devices=None, debug_stage=99):
    cfg = derive_cfg(cfg)
    tiles, npad, nt_tbl, n_tbl = (cfg["tiles"], cfg["npad"], cfg["nt_tbl"],
                                  cfg["n_tbl"])
    TW = 136  # table row width: xl(128) | s_src(4) | s_dst(4)

    nc = bacc.Bacc("TRN2", target_bir_lowering=False, debug=False,
                   num_devices=num_devices or cfg["n_cores"])

    dp = nc.declare_dram_parameter
    x_d = dp("x", [n_tbl, IN_CH], F32, isOutput=False)
    xres_d = dp("xres", [npad, IN_CH], F32, isOutput=False)
    src_d = dp("src_sw", [tiles, P, M], I32, isOutput=False)
    rel_d = dp("rel_sw", [tiles, P, M], F32, isOutput=False)
    relT_d = dp("relT_sw", [tiles, M, P], F32, isOutput=False)
    ea_d = dp("ea_sw", [tiles, P, M, ED], F32, isOutput=False)
    wl_d = dp("W_lin", [HC, IN_CH], F32, isOutput=False)
    we_d = dp("W_edge", [HC, ED], F32, isOutput=False)
    asrc_d = dp("a_src", [HC, 1], F32, isOutput=False)
    adst_d = dp("a_dst", [HC, 1], F32, isOutput=False)
    aedge_d = dp("a_edge", [HC, 1], F32, isOutput=False)
    bias_d = dp("bias", [1, HC], F32, isOutput=False)
    gamma_d = dp("ln_gamma", [1, HC], F32, isOutput=False)
    beta_d = dp("ln_beta", [1, HC], F32, isOutput=False)
    iota_d = dp("iota", [1, P], F32, isOutput=False)
    iotac_d = dp("iota_c", [P, 1], F32, isOutput=False)
    out_d = dp("out", [npad, HC], F32, isOutput=True)

    tbl = nc.dram_tensor("tbl", [n_tbl, TW], F32)
    ct_dram = nc.dram_tensor("ct_scratch", [H, ED], F32)

    with TileContext(nc) as tc:
        with (
            tc.tile_pool(name="const", bufs=1) as cpool,
            tc.tile_pool(name="work", bufs=4) as wpool,
            tc.tile_pool(name="gath", bufs=3) as gpool,
            tc.tile_pool(name="psum", bufs=3, space="PSUM") as pspool,
            tc.tile_pool(name="acc", bufs=2, space="PSUM") as apool,
        ):
            # ---------------- phase A: constants -------------------------
            ident = cpool.tile([P, P], F32, tag="ident")
            make_identity(nc, ident[:])

            wl_sb = cpool.tile([HC, IN_CH], F32, tag="wl")
            nc.sync.dma_start(out=wl_sb[:], in_=wl_d[:])
            we_sb = cpool.tile([HC, ED], F32, tag="we")
            nc.sync.dma_start(out=we_sb[:], in_=we_d[:])
            asrc = cpool.tile([HC, 1], F32, tag="asrc")
            nc.sync.dma_start(out=asrc[:], in_=asrc_d[:])
            adst = cpool.tile([HC, 1], F32, tag="adst")
            nc.sync.dma_start(out=adst[:], in_=adst_d[:])
            aedge = cpool.tile([HC, 1], F32, tag="aedge")
            nc.sync.dma_start(out=aedge[:], in_=aedge_d[:])

            # A8: [128, 8] block-diagonal attention vectors.
            # cols 0:4 -> s_src (uses att_dst), cols 4:8 -> s_dst (att_src)
            a8 = cpool.tile([HC, 2 * H], F32, tag="a8")
            nc.gpsimd.memset(a8[:], 0.0)
            ae4 = cpool.tile([HC, H], F32, tag="ae4")
            nc.gpsimd.memset(ae4[:], 0.0)
            for h in range(H):
                sl = slice(h * C, (h + 1) * C)
                nc.vector.tensor_copy(out=a8[sl, h:h + 1], in_=adst[sl, :])
                nc.vector.tensor_copy(out=a8[sl, H + h:H + h + 1],
                                      in_=asrc[sl, :])
                nc.vector.tensor_copy(out=ae4[sl, h:h + 1], in_=aedge[sl, :])

            # W_lin^T (128x128) and B8 = W_lin^T @ A8 (128x8), fused rhs
            rhsBT = cpool.tile([IN_CH, TW], F32, tag="rhsbt")
            wlT_ps = pspool.tile([P, P], F32, tag="ps")
            nc.tensor.transpose(out=wlT_ps[:], in_=wl_sb[:], identity=ident[:])
            nc.scalar.copy(out=rhsBT[:, 0:HC], in_=wlT_ps[:])
            b8_ps = pspool.tile([IN_CH, 2 * H], F32, tag="ps")
            nc.tensor.matmul(out=b8_ps[:], lhsT=wl_sb[:], rhs=a8[:],
                             start=True, stop=True)
            nc.scalar.copy(out=rhsBT[:, HC:TW], in_=b8_ps[:])

            # C = W_edge^T @ Ae4: [16, 4]; transpose -> [4,16]; broadcast
            c_ps = pspool.tile([ED, H], F32, tag="ps")
            nc.tensor.matmul(out=c_ps[:], lhsT=we_sb[:], rhs=ae4[:],
                             start=True, stop=True)
            c_sb = cpool.tile([ED, H], F32, tag="c_sb")
            nc.vector.tensor_copy(out=c_sb[:], in_=c_ps[:])
            ct_ps = pspool.tile([H, ED], F32, tag="ps")
            nc.tensor.transpose(out=ct_ps[:], in_=c_sb[:],
                                identity=ident[0:ED, 0:ED])
            ct_sb = cpool.tile([H, ED], F32, tag="ct_sb")
            nc.vector.tensor_copy(out=ct_sb[:], in_=ct_ps[:])
            nc.sync.dma_start(out=ct_dram[:], in_=ct_sb[:])
            ctb = cpool.tile([P, H * ED], F32, tag="ctb")
            nc.sync.dma_start(
                out=ctb[:],
                in_=ct_dram[:].rearrange("a b -> (a b)")
                              .unsqueeze(0).to_broadcast([P, H * ED]))

            # broadcast constants: iota row, bias, gamma, beta
            iota_b = cpool.tile([P, P], F32, tag="iota_b")
            nc.sync.dma_start(out=iota_b[:], in_=iota_d[:].to_broadcast([P, P]))
            iota_c = cpool.tile([P, 1], F32, tag="iota_c")
            nc.sync.dma_start(out=iota_c[:], in_=iotac_d[:])
            bias_b = cpool.tile([P, HC], F32, tag="bias_b")
            nc.sync.dma_start(out=bias_b[:], in_=bias_d[:].to_broadcast([P, HC]))
            gamma_b = cpool.tile([P, HC], F32, tag="gamma_b")
            nc.sync.dma_start(out=gamma_b[:],
                              in_=gamma_d[:].to_broadcast([P, HC]))
            beta_b = cpool.tile([P, HC], F32, tag="beta_b")
            nc.sync.dma_start(out=beta_b[:], in_=beta_d[:].to_broadcast([P, HC]))

            eps_t = cpool.tile([P, 1], F32, tag="eps_t")
            nc.gpsimd.memset(eps_t[:], LN_EPS)
            tiny_t = cpool.tile([P, 1], F32, tag="tiny_t")
            nc.gpsimd.memset(tiny_t[:], 1e-16)

            # ---------------- phase B: node table ------------------------
            for t in range(nt_tbl if debug_stage >= 2 else 0):
                xt = wpool.tile([P, IN_CH], F32, tag="xt")
                nc.sync.dma_start(out=xt[:], in_=x_d[t * P:(t + 1) * P, :])
                xT_ps = pspool.tile([P, P], F32, tag="ps")
                nc.tensor.transpose(out=xT_ps[:], in_=xt[:], identity=ident[:])
                xT = wpool.tile([P, P], F32, tag="xT")
                nc.scalar.copy(out=xT[:], in_=xT_ps[:])
                row_ps = pspool.tile([P, TW], F32, tag="ps")
                nc.tensor.matmul(out=row_ps[:], lhsT=xT[:], rhs=rhsBT[:],
                                 start=True, stop=True)
                row = wpool.tile([P, TW], F32, tag="row")
                nc.scalar.copy(out=row[:], in_=row_ps[:])
                nc.scalar.dma_start(out=tbl[t * P:(t + 1) * P, :], in_=row[:])

            # ------------- phase B2: s_dst of own nodes ------------------
            s_own = cpool.tile([P, tiles * H], F32, tag="s_own")
            for t in range(tiles if debug_stage >= 2 else 0):
                xr0 = wpool.tile([P, IN_CH], F32, tag="xt")
                nc.sync.dma_start(out=xr0[:],
                                  in_=xres_d[t * P:(t + 1) * P, :])
                xrT_ps = pspool.tile([P, P], F32, tag="ps")
                nc.tensor.transpose(out=xrT_ps[:], in_=xr0[:],
                                    identity=ident[:])
                xrT = wpool.tile([P, P], F32, tag="xT")
                nc.vector.tensor_copy(out=xrT[:], in_=xrT_ps[:])
                so_ps = pspool.tile([P, H], F32, tag="ps")
                nc.tensor.matmul(out=so_ps[:], lhsT=xrT[:],
                                 rhs=rhsBT[:, HC + H:HC + 2 * H],
                                 start=True, stop=True)
                nc.vector.tensor_copy(out=s_own[:, t * H:(t + 1) * H],
                                      in_=so_ps[:])

            # ---------------- phase C: edges -----------------------------
            n_gb = math.ceil(M / KB)  # gather batches per tile
            for t in range(tiles if debug_stage >= 3 else 0):
                src_t = wpool.tile([P, M], I32, tag="src_t")
                nc.sync.dma_start(out=src_t[:], in_=src_d[t])
                rel_t = wpool.tile([P, M], F32, tag="rel_t")
                nc.sync.dma_start(out=rel_t[:], in_=rel_d[t])
                ea_t = wpool.tile([P, M * ED], F32, tag="ea_t")
                nc.sync.dma_start(
                    out=ea_t[:], in_=ea_d[t].rearrange("p m e -> p (m e)"))

                acc = apool.tile([P, HC + H], F32, tag="acc")

                for gb in range(n_gb):
                    k = min(KB, M - gb * KB)
                    m0 = gb * KB

                    # gathers: single-index-column per call (multi-column
                    # offset APs mis-order on HW)
                    gs = []
                    for j in range(k):
                        m = m0 + j
                        g = gpool.tile([P, 132], F32, tag=f"g{j}")
                        nc.gpsimd.indirect_dma_start(
                            out=g[:], out_offset=None, in_=tbl[:],
                            in_offset=bass.IndirectOffsetOnAxis(
                                ap=src_t[:, m:m + 1], axis=0))
                        gs.append(g)

                    # one-hot matrices for the batch, one DVE op each
                    relT_all = gpool.tile([P, KB * P], F32, tag="relT_all")
                    nc.sync.dma_start(
                        out=relT_all[:, :k * P],
                        in_=relT_d[t, m0:m0 + k].rearrange("m p -> (m p)")
                            .unsqueeze(0).to_broadcast([P, k * P]))
                    rv = relT_all[:].rearrange("p (m q) -> p m q", q=P)
                    oh_all = gpool.tile([P, KB, P], F32, tag="oh_all")
                    nc.vector.tensor_tensor(
                        out=oh_all[:, :k, :],
                        in0=rel_t[:, m0:m0 + k].unsqueeze(2)
                            .to_broadcast([P, k, P]),
                        in1=iota_b[:].unsqueeze(1).to_broadcast([P, k, P]),
                        op=OP.is_equal)
                    ohdt_all = gpool.tile([P, KB, P], F32, tag="ohdt_all")
                    nc.vector.tensor_tensor(
                        out=ohdt_all[:, :k, :],
                        in0=iota_c[:, 0:1].unsqueeze(1)
                            .to_broadcast([P, k, P]),
                        in1=rv[:, :k, :], op=OP.is_equal)

                    # s_dst per edge: one-hot^T(dst) @ s_own_tile on PE
                    sdst_ps = apool.tile([P, KB * H], F32, tag="sdst")
                    for j in range(k):
                        nc.tensor.matmul(
                            out=sdst_ps[:, j * H:(j + 1) * H],
                            lhsT=ohdt_all[:, j, :],
                            rhs=s_own[:, t * H:(t + 1) * H],
                            start=True, stop=True)

                    # alpha8[p, m, h] = edge_attr . C  (batched over chunk)
                    alpha8 = gpool.tile([P, KB, H], F32, tag="alpha8")
                    prod = gpool.tile([P, KB, ED], F32, tag="prod")
                    eav = ea_t[:, m0 * ED:(m0 + k) * ED].rearrange(
                        "p (m e) -> p m e", e=ED)
                    for h in range(H):
                        ctb_h = (ctb[:, h * ED:(h + 1) * ED]
                                 .unsqueeze(1).to_broadcast([P, k, ED]))
                        nc.vector.tensor_tensor(
                            out=prod[:, :k, :], in0=eav, in1=ctb_h,
                            op=OP.mult)
                        nc.vector.reduce_sum(
                            out=alpha8[:, :k, h:h + 1], in_=prod[:, :k, :],
                            axis=AX.X)
                    for j in range(k):
                        nc.vector.tensor_add(out=alpha8[:, j, :],
                                             in0=alpha8[:, j, :],
                                             in1=gs[j][:, HC:HC + H])
                    nc.vector.tensor_add(
                        out=alpha8[:, :k, :], in0=alpha8[:, :k, :],
                        in1=sdst_ps[:, :k * H].rearrange(
                            "p (m h) -> p m h", h=H))
                    ex8 = gpool.tile([P, KB, H], F32, tag="ex8")
                    # leaky_relu(x) = max(x, 0.2*x)
                    nc.vector.scalar_tensor_tensor(
                        out=ex8[:, :k, :], in0=alpha8[:, :k, :],
                        scalar=NEG_SLOPE, in1=alpha8[:, :k, :],
                        op0=OP.mult, op1=OP.max)
                    nc.scalar.activation(out=ex8[:, :k, :],
                                         in_=ex8[:, :k, :], func=AF.Exp)

                    # msg = e * xl_src (+denominator tail)
                    msg_all = gpool.tile([P, KB, HC + H], F32, tag="msg_all")
                    for j in range(k):
                        nc.vector.tensor_tensor(
                            out=msg_all[:, j, 0:HC].rearrange(
                                "p (h c) -> p h c", c=C),
                            in0=gs[j][:, 0:HC].rearrange(
                                "p (h c) -> p h c", c=C),
                            in1=ex8[:, j, :].unsqueeze(2)
                                .to_broadcast([P, H, C]),
                            op=OP.mult)
                    nc.vector.tensor_copy(out=msg_all[:, :k, HC:HC + H],
                                          in_=ex8[:, :k, :])

                    for j in range(k):
                        m = m0 + j
                        nc.tensor.matmul(out=acc[:], lhsT=oh_all[:, j, :],
                                         rhs=msg_all[:, j, :],
                                         start=(m == 0), stop=(m == M - 1))

                # ------------- epilogue for tile t -----------------------
                if debug_stage == 4:
                    o4 = wpool.tile([P, HC], F32, tag="o4")
                    nc.vector.tensor_copy(out=o4[:], in_=acc[:, 0:HC])
                    nc.sync.dma_start(out=out_d[t * P:(t + 1) * P, :],
                                      in_=o4[:])
                    continue
                den = wpool.tile([P, H], F32, tag="den")
                nc.scalar.activation(out=den[:], in_=acc[:, HC:HC + H],
                                     func=AF.Identity, bias=tiny_t[:, 0:1])
                rden = wpool.tile([P, H], F32, tag="rden")
                nc.vector.reciprocal(out=rden[:], in_=den[:])

                o = wpool.tile([P, HC], F32, tag="o")
                for h in range(H):
                    nc.scalar.activation(out=o[:, h * C:(h + 1) * C],
                                         in_=acc[:, h * C:(h + 1) * C],
                                         func=AF.Copy, scale=rden[:, h:h + 1])
                nc.vector.tensor_add(out=o[:], in0=o[:], in1=bias_b[:])
                xr = wpool.tile([P, HC], F32, tag="xr")
                nc.sync.dma_start(out=xr[:], in_=xres_d[t * P:(t + 1) * P, :])
                nc.vector.tensor_add(out=o[:], in0=o[:], in1=xr[:])

                if debug_stage == 5:
                    nc.sync.dma_start(out=out_d[t * P:(t + 1) * P, :],
                                      in_=o[:])
                    continue
                # LayerNorm over channels
                mu = wpool.tile([P, 1], F32, tag="mu")
                nc.vector.reduce_sum(out=mu[:], in_=o[:], axis=AX.X)
                nc.scalar.mul(out=mu[:], in_=mu[:], mul=1.0 / HC)
                ctr = wpool.tile([P, HC], F32, tag="ctr")
                nc.vector.tensor_scalar_sub(out=ctr[:], in0=o[:],
                                            scalar1=mu[:, 0:1])
                sq = wpool.tile([P, HC], F32, tag="sq")
                var = wpool.tile([P, 1], F32, tag="var")
                nc.vector.tensor_mul(out=sq[:], in0=ctr[:], in1=ctr[:])
                nc.vector.reduce_sum(out=var[:], in_=sq[:], axis=AX.X)
                nc.scalar.mul(out=var[:], in_=var[:], mul=1.0 / HC)
                std = wpool.tile([P, 1], F32, tag="std")
                nc.scalar.activation(out=std[:], in_=var[:], func=AF.Sqrt,
                                     bias=eps_t[:, 0:1])
                rstd = wpool.tile([P, 1], F32, tag="rstd")
                nc.vector.reciprocal(out=rstd[:], in_=std[:])
                nrm = wpool.tile([P, HC], F32, tag="nrm")
                nc.scalar.activation(out=nrm[:], in_=ctr[:], func=AF.Copy,
                                     scale=rstd[:, 0:1])
                nc.vector.tensor_mul(out=nrm[:], in0=nrm[:], in1=gamma_b[:])
                nc.vector.tensor_add(out=nrm[:], in0=nrm[:], in1=beta_b[:])

                if debug_stage == 6:
                    nc.sync.dma_start(out=out_d[t * P:(t + 1) * P, :],
                                      in_=nrm[:])
                    continue
                # ELU = relu(x) + min(exp(x)-1, 0)
                ex = wpool.tile([P, HC], F32, tag="ex")
                nc.scalar.activation(out=ex[:], in_=nrm[:], func=AF.Exp)
                nc.vector.tensor_scalar(out=ex[:], in0=ex[:], scalar1=-1.0,
                                        scalar2=0.0, op0=OP.add, op1=OP.min)
                rl = wpool.tile([P, HC], F32, tag="rl")
                nc.scalar.activation(out=rl[:], in_=nrm[:], func=AF.Relu)
                nc.vector.tensor_add(out=rl[:], in0=rl[:], in1=ex[:])
                nc.sync.dma_start(out=out_d[t * P:(t + 1) * P, :], in_=rl[:])

    nc.compile()
    return nc


# --------------------------------------------------------------------------
# entry point
# --------------------------------------------------------------------------

def kernel(**inputs) -> np.ndarray:
    cfg = FULL_CFG
    in_maps, M, dcfg = host_prep(cfg=cfg, **inputs)
    nc = build_program(M, cfg)
    cores = cfg["n_cores"]
    res = run_bass_kernel_spmd(nc, in_maps, list(range(cores)))
    npc = dcfg["npc"]
    parts = [res.results[c]["out"][:npc] for c in range(cores)]
    return np.concatenate(parts, axis=0).astype(np.float32)

